# revision 1
# baseline (speedup 1.0000x reference)
"""Trainium2 Bass kernel for the CerealBar VIN problem.

Self-contained: hardcodes shapes B=512, E=25, 6 orientations, gamma=0.9,
8-core batch sharding (64 samples/core).

Math (bias trick + rescaled domain, derived from the reference):
  The grouped 3x3 conv is a set of one-hot spatial shifts. Encode
  obstacles / out-of-grid cells as a -100 bias folded into the goal map
  (gmB), and iterate in the rescaled domain Y_k = W_k / gamma^(k+1):
    Z[o]   = max(sh_{+d(o)}(Y[o]), sh_{-d(o)}(Y[o]), Y[o+1], Y[o-1])
    Y'[o]  = Z[o] + G_k[o],   G_k = gmB * gamma^-(k+1)  (host-prescaled)
  On free cells all values stay >= 0, so the -100 bias terms never win a
  max and Y tracks the reference's masked W exactly (host multiplies by
  gamma^n and clamps at gather time); masked cells just drift negative.
  Every pass is a plain max or add, all of which run in the DVE's fp16
  2x mode (0.52 ns/elem) -- scalar_tensor_tensor would run at 1x, which
  is why gamma is folded into the streamed G_k tensors instead. 4 vector
  passes/iter (3 max-tree + 1 add), 8 instructions, zero setup compute.
  The 21 G tensors (137 KB/partition) stream from HBM on the sync queue
  in one-iteration chunks issued after each halo pair -- DMA engines
  drain descriptors FIFO, so big chunks would block the halo DMAs.

Device layout: partition p = h*64 + i -> sample i of the core, row-half h.
Each (orientation, half) plane = 21 rows x 26 cols flat (546): row 0 top
halo, rows 1..19 data, row 20 bottom halo, col 25 pad. half0 data = grid
rows 0..18, half1 = grid rows 19..37. Shifts are free-dim AP offsets
(d = 26*dy+dx); the row-split halo rows are refreshed once per iteration
by two cross-partition SBUF-SBUF DMAs kicked right after the boundary
rows (1, 19) of W are built, hidden under ~4us of halo-independent work.
"""
import json
import sys

sys.path.insert(0, "/opt/trn_rl_repo")

import numpy as np

import concourse.bass as bass
import concourse.mybir as mybir
from concourse.ap import AP
from concourse.bass_utils import run_bass_kernel_spmd
from concourse.tile import TileContext

E = 25
ADD = 12
GAMMA = 0.9
BIG = -100.0     # bias for masked cells
PR = 40          # padded full-grid rows (grid rows -1..38 at idx r+1)
PC = 26
SLOT = 546       # 21 * 26 per half-plane
DOF = 26         # data offset within a slot (row 1)
DN = 494         # data elems (rows 1..19)
N_CORES = 8
BPC = 64         # samples per core

# shifts (dy, dx): out[y, x] = in[y+dy, x+dx]
D0 = [(0, 1), (1, 0), (1, -1), (0, -1), (-1, 0), (-1, 1)]
PAIRS = [(0, 3), (1, 2), (4, 5)]  # (0,3) first: dy=0, no halo-row dep

import os as _os

if _os.environ.get("KDT", "fp16") == "fp16":
    DTYPE = mybir.dt.float16
    NP_DT = np.float16
else:
    DTYPE = mybir.dt.float32
    NP_DT = np.float32

TRACE = False
LAST_RESULT = None

_u = np.arange(E)[:, None]
_v = np.arange(E)[None, :]
_ROW = (_u - _v // 2 + ADD) + 1
_COL = np.broadcast_to(_v, (E, E))


# ---------------------------------------------------------------- BIR fixups
def _split_multi_waits(bir):
    """The installed walrus rejects >1 sync wait per instruction; hoist
    extras onto single-wait NoOps inserted before it on the same engine."""
    for fn in bir.get("functions", []):
        for blk in fn.get("blocks", []):
            out = []
            for ins in blk.get("instructions", []):
                si = ins.get("sync_info")
                waits = (si or {}).get("on_wait") or []
                if len(waits) > 1:
                    for k, w in enumerate(waits[:-1]):
                        out.append({
                            "debug": ins.get("debug", 0),
                            "engine": ins["engine"],
                            "ins": [], "outs": [],
                            "name": f"{ins['name']}_w{k}",
                            "opcode": "NoOp",
                            "sync_info": {"on_wait": [w], "on_update": []},
                            "text_hint": "split_wait",
                        })
                    si["on_wait"] = [waits[-1]]
                out.append(ins)
            blk["instructions"] = out
    return bir


def _install_compat(nc):
    orig = nc.to_json_bytes

    def patched():
        return json.dumps(_split_multi_waits(json.loads(orig()))).encode()

    nc.to_json_bytes = patched


# ---------------------------------------------------------------- kernel build
def _rap(t, off, pairs):
    """Raw AP over pool tile t (full 128 partitions) with free dims pairs."""
    return AP(t.tensor, int(t.offset) + off, [list(t.ap[0])] + [list(p) for p in pairs])


def _delta(d):
    return 26 * d[0] + d[1]


def build_nc(n_iter):
    nc = bass.Bass()
    _install_compat(nc)
    mx = mybir.AluOpType.max
    add = mybir.AluOpType.add

    gi_d = nc.declare_dram_parameter("ginit", [128, 6, SLOT], DTYPE, isOutput=False)
    gs_d = nc.declare_dram_parameter("gs", [128, n_iter, 6, SLOT], DTYPE,
                                     isOutput=False)
    w_d = nc.declare_dram_parameter("w", [128, 6, SLOT], DTYPE, isOutput=True)

    with TileContext(nc) as tc:
        with tc.tile_pool(name="p", bufs=1) as pool:
            ginit = pool.tile([128, 6, SLOT], DTYPE)
            gs = pool.tile([128, n_iter, 6, SLOT], DTYPE)
            wb = pool.tile([128, 6, SLOT], DTYPE)
            t0 = pool.tile([128, 6, SLOT], DTYPE)   # X, then Z in place
            t1 = pool.tile([128, 6, SLOT], DTYPE)   # M2

            # Y_{-1} gates iteration 0 -- its DMA goes first, then the first
            # two G tiles. All streaming goes on the sync queue in small
            # per-iteration chunks: DMA engines drain descriptors FIFO, so a
            # big chunk would block the latency-critical halo DMAs behind it.
            nc.sync.dma_start(out=ginit[:, 0:3], in_=gi_d[:, 0:3])
            nc.sync.dma_start(out=ginit[:, 3:6], in_=gi_d[:, 3:6])
            for k in range(min(2, n_iter)):
                nc.sync.dma_start(out=gs[:, k], in_=gs_d[:, k])
            # fake halo rows (grid -1 / grid 38) stay at BIG forever
            nc.gpsimd.memset(wb[0:64, :, 0:26], BIG)
            nc.gpsimd.memset(wb[64:128, :, 20 * 26:21 * 26], BIG)

            def x_pair(src, oa, ob):
                # t0[{oa,ob}] = max(sh_{+d0}(src), sh_{-d0}(src))
                da, db = _delta(D0[oa]), _delta(D0[ob])
                step = (ob - oa) * SLOT
                in0 = _rap(src, oa * SLOT + DOF + da, [[step + (db - da), 2], [1, DN]])
                in1 = _rap(src, oa * SLOT + DOF - da, [[step - (db - da), 2], [1, DN]])
                out = _rap(t0, oa * SLOT + DOF, [[step, 2], [1, DN]])
                nc.vector.tensor_tensor(out=out, in0=in0, in1=in1, op=mx)

            dv = (slice(None), slice(None), slice(DOF, DOF + DN))

            def halo():
                # half1 top halo <- half0 grid row 18 (buffer row 19)
                nc.sync.dma_start(out=wb[64:128, :, 0:26],
                                  in_=wb[0:64, :, 19 * 26:20 * 26])
                # half0 bottom halo <- half1 grid row 19 (buffer row 1)
                nc.sync.dma_start(out=wb[0:64, :, 20 * 26:21 * 26],
                                  in_=wb[64:128, :, 26:52])

            for it in range(n_iter):
                src = ginit if it == 0 else wb
                # halo-independent ops first so the previous iteration's
                # halo DMA has the whole window to land. On iteration 0,
                # X(1,2) goes first: it only needs the first ginit chunk.
                if it == 0:
                    x_pair(src, 1, 2)
                    x_pair(src, 0, 3)
                else:
                    x_pair(src, 0, 3)
                # M2: middle slots batched, then wrap slots {0,5}
                nc.vector.tensor_tensor(
                    out=t1[:, 1:5, DOF:DOF + DN], in0=src[:, 2:6, DOF:DOF + DN],
                    in1=src[:, 0:4, DOF:DOF + DN], op=mx)
                nc.vector.tensor_tensor(
                    out=_rap(t1, DOF, [[5 * SLOT, 2], [1, DN]]),
                    in0=_rap(src, SLOT + DOF, [[-SLOT, 2], [1, DN]]),
                    in1=_rap(src, 5 * SLOT + DOF, [[-SLOT, 2], [1, DN]]), op=mx)
                if it == 0:
                    x_pair(src, 4, 5)
                else:
                    x_pair(src, 1, 2)
                    x_pair(src, 4, 5)
                # Z = max(X, M2) in place in t0
                nc.vector.tensor_tensor(out=t0[dv], in0=t0[dv], in1=t1[dv], op=mx)
                # Y' = Z + G_it (pure adds at fp16 2x rate); boundary rows
                # (1, 19) first so the halo DMAs overlap the interior build
                goff = it * 6 * SLOT
                if it < n_iter - 1:
                    wa = _rap(wb, 26, [[SLOT, 6], [468, 2], [1, 26]])
                    za = _rap(t0, 26, [[SLOT, 6], [468, 2], [1, 26]])
                    ga = _rap(gs, goff + 26, [[SLOT, 6], [468, 2], [1, 26]])
                    nc.vector.tensor_tensor(out=wa, in0=za, in1=ga, op=add)
                    halo()
                    # stream the G tile two iterations ahead, after the halo
                    # DMAs so it can never delay them in the engine FIFOs
                    if it + 2 < n_iter:
                        nc.sync.dma_start(out=gs[:, it + 2], in_=gs_d[:, it + 2])
                    wi = _rap(wb, 52, [[SLOT, 6], [1, 442]])
                    zi = _rap(t0, 52, [[SLOT, 6], [1, 442]])
                    gi = _rap(gs, goff + 52, [[SLOT, 6], [1, 442]])
                    nc.vector.tensor_tensor(out=wi, in0=zi, in1=gi, op=add)
                else:
                    # final iteration: no halo to kick, so skip the
                    # boundary split -- one contiguous rows-1..19 add per
                    # slot half, each half shipped as soon as it is built
                    for s0 in (0, 3):
                        wi = _rap(wb, s0 * SLOT + 26, [[SLOT, 3], [1, 494]])
                        zi = _rap(t0, s0 * SLOT + 26, [[SLOT, 3], [1, 494]])
                        gi = _rap(gs, goff + s0 * SLOT + 26, [[SLOT, 3], [1, 494]])
                        nc.vector.tensor_tensor(out=wi, in0=zi, in1=gi, op=add)
                        eng = nc.sync if s0 == 0 else nc.scalar
                        eng.dma_start(out=w_d[:, s0:s0 + 3], in_=wb[:, s0:s0 + 3])
    return nc


_NC_CACHE = {}


def _get_nc(n_iter):
    if n_iter not in _NC_CACHE:
        _NC_CACHE[n_iter] = build_nc(n_iter)
    return _NC_CACHE[n_iter]


# ---------------------------------------------------------------- host side
def _to_padded_axial(x):
    out = np.zeros(x.shape[:-2] + (PR, PC), np.float32)
    out[..., _ROW, _COL] = x
    return out


def kernel(offset_input_goals, offset_current_state, offset_obstacles,
           num_iterations):
    global LAST_RESULT
    goals = np.asarray(offset_input_goals, np.float32)
    state = np.asarray(offset_current_state)
    obst = np.asarray(offset_obstacles, np.float32)
    n_iter = int(num_iterations)
    B = goals.shape[0]
    assert B == N_CORES * BPC and n_iter >= 1

    goals_ax = _to_padded_axial(goals)                     # [B,6,40,26]
    mask = _to_padded_axial(np.ones((E, E), np.float32))
    m_full = (1.0 - _to_padded_axial(obst)) * mask         # [B,40,26]
    gmb_full = np.where(m_full[:, None] > 0.5, goals_ax, BIG)  # [B,6,40,26]

    def split(x):  # [B, ..., 40, 26] -> [B, ..., 546] halves
        h0 = x[..., 0:21, :].reshape(x.shape[:-2] + (SLOT,))
        h1 = x[..., 19:40, :].reshape(x.shape[:-2] + (SLOT,))
        return h0, h1

    g0, g1 = split(gmb_full)
    gmb_h = np.stack([g0, g1], 1)                          # [B,2,6,546]
    # rescaled-domain goal tensors: Y_k = W_k / gamma^(k+1) turns the
    # update into  Y' = max-tree(Y) + G_k,  G_k = gmb * gamma^-(k+1)
    scales = GAMMA ** -(np.arange(1, n_iter + 1, dtype=np.float32))
    gs_h = (gmb_h[:, :, None] * scales[None, None, :, None, None]).astype(NP_DT)

    in_maps = []
    for c in range(N_CORES):
        s = slice(c * BPC, (c + 1) * BPC)
        gi = np.concatenate([g0[s], g1[s]], 0).astype(NP_DT)
        gsc = np.concatenate([gs_h[s, 0], gs_h[s, 1]], 0)  # [128,n_iter,6,546]
        in_maps.append({"ginit": gi, "gs": gsc})

    nc = _get_nc(n_iter)
    res = run_bass_kernel_spmd(nc, in_maps, core_ids=list(range(N_CORES)),
                               trace=TRACE)
    LAST_RESULT = res

    w_all = np.stack([np.asarray(res.results[c]["w"], np.float32)
                      for c in range(N_CORES)], 0)         # [8,128,6,546]

    alpha = state[:, 0].astype(np.int64)
    uu = (state[:, 1] - state[:, 2] // 2 + ADD).astype(np.int64)  # grid row
    vv = state[:, 2].astype(np.int64)
    rot = (alpha + 1) % 6
    bs = np.arange(B)
    core = bs // BPC
    lane = bs % BPC

    w_scale = np.float32(GAMMA ** n_iter)   # W_final = Y_final * gamma^n

    def read_w(slot, g, c):
        # clamped gather of W at grid row g, col c (0 outside grid / masked)
        valid = (g >= 0) & (g <= 37) & (c >= 0) & (c <= 24)
        h = (g > 18).astype(np.int64)
        local = np.where(h == 1, g - 18, g + 1)
        p = h * 64 + lane
        idx = np.clip(local * 26 + c, 0, SLOT - 1)
        val = w_all[core, p, slot, idx] * w_scale
        return np.where(valid, np.maximum(val, 0.0), 0.0)

    dy0 = np.array([d[0] for d in D0])[rot]
    dx0 = np.array([d[1] for d in D0])[rot]
    m_pt = m_full[bs, uu + 1, vv]

    out = np.zeros((B, 4), np.float32)
    out[:, 0] = m_pt * read_w(rot, uu + dy0, vv + dx0)
    out[:, 1] = m_pt * read_w(rot, uu - dy0, vv - dx0)
    out[:, 2] = read_w((rot + 1) % 6, uu, vv)
    out[:, 3] = read_w((rot + 5) % 6, uu, vv)
    return out



# revision 3
# speedup vs baseline: 1.2109x; 1.2109x over previous
"""Trainium2 Bass kernel for the CerealBar VIN problem — offset-coords layout.

Self-contained: hardcodes shapes B=512, E=25, 6 orientations, gamma=0.9,
8-core batch sharding (64 samples/core).

Math: the VIN update per orientation o is
    v'[o] = max(sh_{+d(o)} v[o], sh_{-d(o)} v[o], v[o+1], v[o-1])
    Y'    = v' + G_k,   G_k = gmB * gamma^-(k+1)   (rescaled domain,
            gmB = goals with -100 at obstacles/out-of-grid; host prescales)
Unlike the reference (and the previous kernel revision), iteration happens in
ORIGINAL OFFSET coordinates (25x25, every cell valid) instead of the 37x25
axial embedding: hex-neighbor shifts become column-parity-dependent storage
offsets, handled by splitting the two dv!=0 shift passes into even/odd-column
instructions.  This cuts DVE work per pass from 988 to 650 cols/plane (-34%).

Device layout: partition p = h*64 + i -> sample i, column-half h.
Per orientation slot: 15 col-positions x 27 rows, column-major (SLOT=405).
half0 stores v=-1..13 at jj=0..14 (jj=0 dead BIG, jj=14 = halo col v=13);
half1 stores v=11..25 (jj=0 = halo col v=11, jj=14 dead).  Column v=12 is
computed redundantly by both halves so storage-parity == v-parity in both
(required for lockstep parity-split instructions).  Rows ss=0/26 are BIG
borders.  All constants (borders, dead cols, initial halos) are host-baked
into the initial W load; no device memsets.

Halo: one column per direction per iteration.  Z for the owner columns is
computed early into a staging tile, DMA'd cross-half into the peer's halo
column of wb (sync + scalar queues), and the +G add for the halo column runs
at the START of the next iteration (after the dv=0 passes), so the DMA has a
~3.4us window and the DVE never stalls on it.
"""
import json
import sys

sys.path.insert(0, "/opt/trn_rl_repo")

import numpy as np

import concourse.bass as bass
import concourse.mybir as mybir
from concourse.ap import AP
from concourse.bass_utils import run_bass_kernel_spmd
from concourse.tile import TileContext

E = 25
GAMMA = 0.9
BIG = -100.0
C = 27            # rows per column (u=-1..25)
NCOL = 15         # col-positions per slot (jj=0..14)
S = C * NCOL      # 405 elems per slot
PLANE = 6 * S     # 2430 per partition
N_CORES = 8
BPC = 64

# axial-basis hex directions per orientation
D_AX = [(0, 1), (1, 0), (1, -1), (0, -1), (-1, 0), (-1, 1)]
PAIRS = [(1, 4), (0, 3), (2, 5)]   # antipodal orientation pairs

import os as _os

if _os.environ.get("KDT", "fp16") == "fp16":
    DTYPE = mybir.dt.float16
    NP_DT = np.float16
else:
    DTYPE = mybir.dt.float32
    NP_DT = np.float32

TRACE = False
LAST_RESULT = None


def _delta(o, par):
    """Storage delta (elements) for shifting by hex dir o at columns of
    v-parity par (0=even).  ds = dr + ((v+dv)//2 - v//2), delta = ds + dv*C."""
    dr, dv = D_AX[o]
    du = dr + ((par + dv) // 2 - par // 2)
    return du + dv * C


# ---------------------------------------------------------------- BIR fixups
def _split_multi_waits(bir):
    """The installed walrus rejects >1 sync wait per instruction; hoist
    extras onto single-wait NoOps inserted before it on the same engine."""
    for fn in bir.get("functions", []):
        for blk in fn.get("blocks", []):
            out = []
            for ins in blk.get("instructions", []):
                si = ins.get("sync_info")
                waits = (si or {}).get("on_wait") or []
                if len(waits) > 1:
                    for k, w in enumerate(waits[:-1]):
                        out.append({
                            "debug": ins.get("debug", 0),
                            "engine": ins["engine"],
                            "ins": [], "outs": [],
                            "name": f"{ins['name']}_w{k}",
                            "opcode": "NoOp",
                            "sync_info": {"on_wait": [w], "on_update": []},
                            "text_hint": "split_wait",
                        })
                    si["on_wait"] = [waits[-1]]
                out.append(ins)
            blk["instructions"] = out
    return bir


def _install_compat(nc):
    orig = nc.to_json_bytes

    def patched():
        return json.dumps(_split_multi_waits(json.loads(orig()))).encode()

    nc.to_json_bytes = patched


# ---------------------------------------------------------------- kernel build
def _rap(t, off, pairs):
    """Raw AP over pool tile t (full 128 partitions) with free dims pairs."""
    return AP(t.tensor, int(t.offset) + off,
              [list(t.ap[0])] + [list(p) for p in pairs])


def _raph(t, half, off, pairs):
    """Raw AP over one 64-partition half of pool tile t."""
    base = t[64:128] if half else t[0:64]
    return AP(t.tensor, int(base.offset) + off,
              [list(base.ap[0])] + [list(p) for p in pairs])


def build_nc(n_iter):
    nc = bass.Bass()
    _install_compat(nc)
    mx = mybir.AluOpType.max
    add = mybir.AluOpType.add

    wi_d = nc.declare_dram_parameter("winit", [128, 6, S], DTYPE, isOutput=False)
    gs_d = nc.declare_dram_parameter("gs", [128, n_iter, 6, S], DTYPE,
                                     isOutput=False)
    w_d = nc.declare_dram_parameter("w", [128, 6, S], DTYPE, isOutput=True)

    with TileContext(nc) as tc:
        with tc.tile_pool(name="p", bufs=1) as pool:
            wb = pool.tile([128, 6, S], DTYPE)
            gs = pool.tile([128, n_iter, 6, S], DTYPE)
            t0 = pool.tile([128, 6, S], DTYPE)   # X, then Z
            t1 = pool.tile([128, 6, S], DTYPE)   # M2
            hs = pool.tile([128, 6, 32], DTYPE)  # halo staging (Z of owner col)

            # initial W (gmB incl borders/halos) split across two queues,
            # then the first G chunks.
            nc.sync.dma_start(out=wb[:, 0:3], in_=wi_d[:, 0:3])
            nc.scalar.dma_start(out=wb[:, 3:6], in_=wi_d[:, 3:6])
            for k in range(min(2, n_iter)):
                eng = nc.sync if k == 0 else nc.scalar
                eng.dma_start(out=gs[:, k], in_=gs_d[:, k])

            def x_pair(a, b, par):
                """t0[{a,b}] = max(sh_{+d} wb, sh_{-d} wb) on par-parity cols."""
                jj0, ncols = (1, 7) if par == 0 else (2, 6)
                da, db = _delta(a, par), _delta(b, par)
                base = a * S + jj0 * C + 1
                step = (b - a) * S
                out = _rap(t0, base, [[step, 2], [2 * C, ncols], [1, 25]])
                in0 = _rap(wb, base + da,
                           [[step + (db - da), 2], [2 * C, ncols], [1, 25]])
                in1 = _rap(wb, base + db,
                           [[step + (da - db), 2], [2 * C, ncols], [1, 25]])
                nc.vector.tensor_tensor(out=out, in0=in0, in1=in1, op=mx)

            def x_pair14():
                """dv=0 pair (1,4): parity-free, all 13 cols in one go."""
                base = S + C + 1
                out = _rap(t0, base, [[3 * S, 2], [C, 13], [1, 25]])
                in0 = _rap(wb, base + 1, [[3 * S - 2, 2], [C, 13], [1, 25]])
                in1 = _rap(wb, base - 1, [[3 * S + 2, 2], [C, 13], [1, 25]])
                nc.vector.tensor_tensor(out=out, in0=in0, in1=in1, op=mx)

            dcols = [[C, 13], [1, 25]]   # data cols jj=1..13, rows 1..25

            def addH(it):
                """+G for the DMA-landed halo cols of iteration `it` (in wb)."""
                goff = it * PLANE
                # half0: halo col jj=14 (v=13)
                nc.vector.tensor_tensor(
                    out=_raph(wb, 0, 14 * C + 1, [[S, 6], [1, 25]]),
                    in0=_raph(wb, 0, 14 * C + 1, [[S, 6], [1, 25]]),
                    in1=_raph(gs, 0, goff + 14 * C + 1, [[S, 6], [1, 25]]),
                    op=add)
                # half1: halo col jj=0 (v=11)
                nc.vector.tensor_tensor(
                    out=_raph(wb, 1, 1, [[S, 6], [1, 25]]),
                    in0=_raph(wb, 1, 1, [[S, 6], [1, 25]]),
                    in1=_raph(gs, 1, goff + 1, [[S, 6], [1, 25]]),
                    op=add)

            for it in range(n_iter):
                # ---- passes that need no halo columns
                x_pair14()
                # M2 mid: t1[1:5] = max(wb[2:6], wb[0:4])
                nc.vector.tensor_tensor(
                    out=_rap(t1, S + C + 1, [[S, 4]] + dcols),
                    in0=_rap(wb, 2 * S + C + 1, [[S, 4]] + dcols),
                    in1=_rap(wb, C + 1, [[S, 4]] + dcols), op=mx)
                # M2 wrap: t1[0] = max(wb1, wb5); t1[5] = max(wb0, wb4)
                nc.vector.tensor_tensor(
                    out=_rap(t1, C + 1, [[5 * S, 2]] + dcols),
                    in0=_rap(wb, S + C + 1, [[-S, 2]] + dcols),
                    in1=_rap(wb, 5 * S + C + 1, [[-S, 2]] + dcols), op=mx)
                # ---- halo-col +G for previous iteration (DMA landed long ago)
                if it > 0:
                    addH(it - 1)
                # ---- parity-split shift pairs (read halo cols)
                x_pair(0, 3, 0)
                x_pair(0, 3, 1)
                x_pair(2, 5, 0)
                x_pair(2, 5, 1)
                # ---- early Z of halo-owner cols -> staging -> cross-half DMA
                if it < n_iter - 1:
                    # half0 owns v=11 (jj=12) for half1's halo
                    nc.vector.tensor_tensor(
                        out=_raph(hs, 0, 0, [[32, 6], [1, 25]]),
                        in0=_raph(t0, 0, 12 * C + 1, [[S, 6], [1, 25]]),
                        in1=_raph(t1, 0, 12 * C + 1, [[S, 6], [1, 25]]), op=mx)
                    # half1 owns v=13 (jj=2) for half0's halo
                    nc.vector.tensor_tensor(
                        out=_raph(hs, 1, 0, [[32, 6], [1, 25]]),
                        in0=_raph(t0, 1, 2 * C + 1, [[S, 6], [1, 25]]),
                        in1=_raph(t1, 1, 2 * C + 1, [[S, 6], [1, 25]]), op=mx)
                    nc.sync.dma_start(
                        out=_raph(wb, 0, 14 * C + 1, [[S, 6], [1, 25]]),
                        in_=_raph(hs, 1, 0, [[32, 6], [1, 25]]))
                    nc.scalar.dma_start(
                        out=_raph(wb, 1, 1, [[S, 6], [1, 25]]),
                        in_=_raph(hs, 0, 0, [[32, 6], [1, 25]]))
                # ---- Z over all data cols (in place in t0)
                nc.vector.tensor_tensor(
                    out=_rap(t0, C + 1, [[S, 6]] + dcols),
                    in0=_rap(t0, C + 1, [[S, 6]] + dcols),
                    in1=_rap(t1, C + 1, [[S, 6]] + dcols), op=mx)
                # ---- Y' = Z + G_it
                goff = it * PLANE
                if it < n_iter - 1:
                    nc.vector.tensor_tensor(
                        out=_rap(wb, C + 1, [[S, 6]] + dcols),
                        in0=_rap(t0, C + 1, [[S, 6]] + dcols),
                        in1=_rap(gs, goff + C + 1, [[S, 6]] + dcols), op=add)
                    if it + 2 < n_iter:
                        eng = nc.sync if it % 2 == 0 else nc.scalar
                        eng.dma_start(out=gs[:, it + 2], in_=gs_d[:, it + 2])
                else:
                    # final iteration: add + ship per slot-triplet, overlapped
                    for s0 in (0, 3):
                        nc.vector.tensor_tensor(
                            out=_rap(wb, s0 * S + C + 1, [[S, 3]] + dcols),
                            in0=_rap(t0, s0 * S + C + 1, [[S, 3]] + dcols),
                            in1=_rap(gs, goff + s0 * S + C + 1,
                                     [[S, 3]] + dcols), op=add)
                        eng = nc.sync if s0 == 0 else nc.scalar
                        eng.dma_start(out=w_d[:, s0:s0 + 3],
                                      in_=wb[:, s0:s0 + 3])
    return nc


_NC_CACHE = {}


def _get_nc(n_iter):
    if n_iter not in _NC_CACHE:
        _NC_CACHE[n_iter] = build_nc(n_iter)
    return _NC_CACHE[n_iter]


# ---------------------------------------------------------------- host side
def kernel(offset_input_goals, offset_current_state, offset_obstacles,
           num_iterations):
    global LAST_RESULT
    goals = np.asarray(offset_input_goals, np.float32)
    state = np.asarray(offset_current_state)
    obst = np.asarray(offset_obstacles, np.float32)
    n_iter = int(num_iterations)
    B = goals.shape[0]
    assert B == N_CORES * BPC and n_iter >= 1

    m = 1.0 - obst                                          # [B,25,25] free
    gmb = np.where(m[:, None] > 0.5, goals, BIG)            # [B,6,25,25]
    P = np.full((B, 6, E + 2, E + 2), BIG, np.float32)      # [u+1, v+1]
    P[:, :, 1:26, 1:26] = gmb

    # column-major halves: [B, 6, 15(jj), 27(ss)]
    H0 = P[:, :, :, 0:15].swapaxes(2, 3).reshape(B, 6, S)
    H1 = P[:, :, :, 12:27].swapaxes(2, 3).reshape(B, 6, S)
    gh = np.stack([H0, H1], 1)                              # [B,2,6,S]
    scales = GAMMA ** -(np.arange(1, n_iter + 1, dtype=np.float32))
    gs_all = (gh[:, :, None] * scales[None, None, :, None, None]).astype(NP_DT)

    in_maps = []
    for c in range(N_CORES):
        s = slice(c * BPC, (c + 1) * BPC)
        wi = np.concatenate([H0[s], H1[s]], 0).astype(NP_DT)   # [128,6,S]
        gsc = np.concatenate([gs_all[s, 0], gs_all[s, 1]], 0)  # [128,n,6,S]
        in_maps.append({"winit": wi, "gs": gsc})

    nc = _get_nc(n_iter)
    res = run_bass_kernel_spmd(nc, in_maps, core_ids=list(range(N_CORES)),
                               trace=TRACE)
    LAST_RESULT = res

    w_all = np.stack([np.asarray(res.results[c]["w"], np.float32)
                      for c in range(N_CORES)], 0)          # [8,128,6,S]
    w_all = w_all.reshape(8, 2, BPC, 6, NCOL, C)
    wh0 = w_all[:, 0].reshape(B, 6, NCOL, C)
    wh1 = w_all[:, 1].reshape(B, 6, NCOL, C)
    W = np.empty((B, 6, E, E), np.float32)                  # [u, v]
    W[:, :, :, 0:13] = wh0[:, :, 1:14, 1:26].transpose(0, 1, 3, 2)
    W[:, :, :, 13:25] = wh1[:, :, 2:14, 1:26].transpose(0, 1, 3, 2)

    alpha = state[:, 0].astype(np.int64)
    u = state[:, 1].astype(np.int64)
    v = state[:, 2].astype(np.int64)
    rot = (alpha + 1) % 6
    bs = np.arange(B)
    w_scale = np.float32(GAMMA ** n_iter)

    def read_w(slot, uu, vv):
        valid = (uu >= 0) & (uu < E) & (vv >= 0) & (vv < E)
        uc = np.clip(uu, 0, E - 1)
        vc = np.clip(vv, 0, E - 1)
        val = W[bs, slot, uc, vc] * w_scale
        return np.where(valid, np.maximum(val, 0.0), 0.0)

    dr = np.array([d[0] for d in D_AX])[rot]
    dv = np.array([d[1] for d in D_AX])[rot]
    ds_f = dr + ((v + dv) // 2 - v // 2)
    ds_b = -dr + ((v - dv) // 2 - v // 2)
    m_pt = m[bs, u, v]

    out = np.zeros((B, 4), np.float32)
    out[:, 0] = m_pt * read_w(rot, u + ds_f, v + dv)
    out[:, 1] = m_pt * read_w(rot, u + ds_b, v - dv)
    out[:, 2] = read_w((rot + 1) % 6, u, v)
    out[:, 3] = read_w((rot + 5) % 6, u, v)
    return out
